# revision 19
# baseline (speedup 1.0000x reference)
"""Trainium2 Bass kernel for nn_LCN (locally-connected network).

Computation:
  x: (512, 1, 280, 280) -> non-overlapping 28x28 patches (10x10 grid, P=100)
  y[b, f, p] = sum_q x[b, p, q] * w[f*100+p, q]    (q = k*28+l, 784 per patch)
  y = relu(y + bias[f*100+p]);  out = y_flat @ dec_w.T + dec_b  (j = f*100+p)

Sharding: patch-parallel. All 8 cores see all 512 images; core c owns 13
patches (cores 4-7 own 12 real + 1 zero patch so every core runs the same
program). Per core:
  - host stages x TRANSPOSED as xT [128 px, 80 chunks, 512 b] bf16
    (im2col + transpose + cast all on host; DMA reads are contiguous
    multi-KB runs per partition at full HBM bandwidth)
  - conv: one matmul per (128-px chunk, patch-pair window): lhsT = staged
    weight tile [128, 32], rhs = xT[:, t, :] [128, 512] -> PSUM [128, 512]
    accumulating per 8-patch group (partition j = 16*local_patch + f)
  - ACT: relu(psum + bias) -> y_sb
  - decoder: 2 accumulating matmuls lhsT=dec [K, 10] -> out [10, 512]
Host sums the 8 per-core partial decoder outputs and adds dec_b.
"""

import sys

import numpy as np

for _p in ("/opt/trn_rl_repo", "/opt/trn_rl_repo/concourse"):
    if _p not in sys.path:
        sys.path.insert(0, _p)

import concourse.bass as bass
import concourse.mybir as mybir
import concourse.tile as tile
from concourse import bacc

F32 = mybir.dt.float32
BF16 = mybir.dt.bfloat16

# Problem constants
B = 512
P = 100
F = 16
OUT = 10
PPX = 784            # pixels per patch (28*28)
NCORES = 8

NPAT = 13            # patches per core program (zero-padded on 12-patch cores)
NCHUNK = 80          # ceil(13*784/128)
PXPAD = NCHUNK * 128  # 10240
GROUPS = [(0, 8), (8, 5)]   # (first local patch, n patches incl virtual pad)
GROUP_CHUNKS = [(0, 49), (49, 31)]  # (first chunk, n chunks); 8*784 = 49*128
# small first split to warm up PE early, small last split to trim the tail
_SPLIT_SIZES = [4, 8, 12, 13, 13, 13, 13, 4]
DMA_SPLITS = []
_c = 0
for _s in _SPLIT_SIZES:
    DMA_SPLITS.append((_c, _s))
    _c += _s
assert _c == NCHUNK

# per-core real patch ranges (cores 0-3: 13 patches, cores 4-7: 12)
CORE_PSTART = [0, 13, 26, 39, 52, 64, 76, 88]
CORE_NPAT = [13, 13, 13, 13, 12, 12, 12, 12]


def conv_plan():
    """Static matmul plan: one entry per (chunk, patch-pair window)."""
    plan = []
    for t in range(NCHUNK):
        p0 = (128 * t) // PPX
        p1 = (128 * t + 127) // PPX
        g = p0 // 8
        pairs = sorted({(min(p, 8 * g + 9) - 8 * g) // 2 for p in (p0, p1)})
        for k in pairs:
            first = ((8 * g + 2 * k) * PPX) // 128
            last = min(((8 * g + 2 * k + 2) * PPX - 1) // 128, NCHUNK - 1)
            plan.append((t, g, k, t == first, t == last))
    return plan

PLAN = conv_plan()
NMM = len(PLAN)  # 85


def build_program():
    nc = bacc.Bacc("TRN2")
    x_d = nc.dram_tensor("x", [128, NCHUNK * B], BF16, kind="ExternalInput")
    # w tiles then decoder staged in one tensor / one DMA
    w_d = nc.dram_tensor("w", [128, NMM * 32 + 2 * OUT], BF16,
                         kind="ExternalInput")
    b_d = nc.dram_tensor("bias", [128, 2], F32, kind="ExternalInput")
    o_d = nc.dram_tensor("out", [OUT, B], F32, kind="ExternalOutput")

    with tile.TileContext(nc) as tc:
        with (
            tc.tile_pool(name="const", bufs=1) as constp,
            tc.tile_pool(name="yps", bufs=2, space="PSUM") as ypsp,
            tc.tile_pool(name="wps", bufs=1, space="PSUM") as wpsp,
            tc.tile_pool(name="ops", bufs=1, space="PSUM") as opsp,
        ):
            w_sb = constp.tile([128, NMM * 32 + 2 * OUT], BF16)
            nc.scalar.dma_start(out=w_sb[:], in_=w_d[:])
            dec_sb = w_sb[:, NMM * 32:]
            bias_sb = constp.tile([128, 2], F32)
            nc.scalar.dma_start(out=bias_sb[:], in_=b_d[:])

            xt = constp.tile([128, NCHUNK, B], BF16)
            for c0, n in DMA_SPLITS:
                nc.sync.dma_start(
                    out=xt[:, c0:c0 + n, :],
                    in_=x_d[:, c0 * B:(c0 + n) * B],
                )

            # PE clock warm-up during the DMA pipe-fill window: M=128 dummy
            # matmuls on an uninitialized scratch tile (no input dependency,
            # output never read)
            warm_sb = constp.tile([128, 128 + B], BF16)
            nc.gpsimd.memset(warm_sb[:], 0.0)
            warm_ps = wpsp.tile([128, B], F32)
            for _ in range(9):
                nc.tensor.matmul(
                    warm_ps[:],
                    warm_sb[:, 0:128],
                    warm_sb[:, 128:],
                    start=True,
                    stop=True,
                )

            y_sb = constp.tile([128, 2, B], BF16)
            ps = [ypsp.tile([128, B], F32, name=f"ps{g}") for g in range(2)]
            out_ps = opsp.tile([OUT, B], F32)

            rows = [16 * 8, 16 * 5]  # evacuated rows per group
            for g in range(2):
                gi = [e for e in enumerate(PLAN) if e[1][1] == g]
                for i, (t, _, k, st, sp) in gi:
                    nc.tensor.matmul(
                        ps[g][32 * k:32 * k + 32, :],
                        w_sb[:, 32 * i:32 * i + 32],
                        xt[:, t, :],
                        start=st,
                        stop=sp,
                        tile_position=(0, 32 * k),
                    )
                nc.scalar.activation(
                    out=y_sb[0:rows[g], g, :],
                    in_=ps[g][0:rows[g], :],
                    func=mybir.ActivationFunctionType.Relu,
                    bias=bias_sb[0:rows[g], g:g + 1],
                )
                nc.tensor.matmul(
                    out_ps[:],
                    dec_sb[0:rows[g], g * OUT:(g + 1) * OUT],
                    y_sb[0:rows[g], g, :],
                    start=(g == 0),
                    stop=(g == 1),
                )
            out_sb = constp.tile([OUT, B], F32)
            nc.vector.tensor_copy(out_sb[:], out_ps[:])
            nc.sync.dma_start(out=o_d[:], in_=out_sb[:])

    return nc


def stage_core(core, x_pm, weight, bias, dec_w):
    """Host-side staging for one core. x_pm: (B, 100, 784) float32."""
    import ml_dtypes

    p0 = CORE_PSTART[core]
    npr = CORE_NPAT[core]
    pids = list(range(p0, p0 + npr))

    xs = np.zeros((B, PXPAD), np.float32)
    xs[:, :npr * PPX] = x_pm[:, p0:p0 + npr, :].reshape(B, npr * PPX)
    # host-side transpose to [px_part 128, chunk, batch], bf16
    xs = np.ascontiguousarray(
        xs.reshape(B, NCHUNK, 128).transpose(2, 1, 0)
    ).astype(ml_dtypes.bfloat16).reshape(128, NCHUNK * B)

    wr = np.asarray(weight, np.float32).reshape(F, P, PPX)
    w_big = np.zeros((128, NMM * 32), np.float32)
    for i, (t, g, k, _, _) in enumerate(PLAN):
        for r in range(128):
            px = 128 * t + r
            p = px // PPX
            if p >= npr:
                continue
            pl = p - 8 * g
            if pl < 0 or pl // 2 != k:
                continue
            q = px % PPX
            w_big[r, 32 * i + (pl % 2) * 16:32 * i + (pl % 2) * 16 + F] = \
                wr[:, pids[p], q]

    br = np.asarray(bias, np.float32).reshape(F, P)
    dr = np.asarray(dec_w, np.float32).reshape(OUT, F, P)
    b_st = np.zeros((128, 2), np.float32)
    d_st = np.zeros((128, 2 * OUT), np.float32)
    for p in range(npr):
        g, pl = p // 8, p % 8
        j = 16 * pl + np.arange(F)
        b_st[j, g] = br[:, pids[p]]
        d_st[j[:, None], g * OUT + np.arange(OUT)[None, :]] = dr[:, :, pids[p]].T
    w_all = np.concatenate(
        [w_big, d_st], axis=1).astype(ml_dtypes.bfloat16)
    return {"x": xs, "w": w_all, "bias": b_st}


_cache = {}


def _get_nc():
    if "nc" not in _cache:
        nc = build_program()
        nc.finalize()
        _cache["nc"] = nc
    return _cache["nc"]


def make_in_maps(x, weight, bias, dec_w):
    x = np.asarray(x, np.float32)
    # patch-major pixel order: (b, ph, pw, k, l)
    x_pm = np.ascontiguousarray(
        x.reshape(B, 10, 28, 10, 28).transpose(0, 1, 3, 2, 4)
    ).reshape(B, P, PPX)
    return [stage_core(c, x_pm, weight, bias, dec_w) for c in range(NCORES)]


def combine(results, dec_b):
    acc = np.zeros((OUT, B), np.float32)
    for r in results:
        acc += r["out"]
    return acc.T + np.asarray(dec_b, np.float32)


def _install_ntff_hook():
    """Provide the missing antenv.axon_hooks module so trace=True works
    under axon (replicates trn_boot._ntff_profile_via_ctypes)."""
    import contextlib
    import ctypes
    import types

    if "antenv.axon_hooks" in sys.modules:
        return
    so_path = "/opt/axon/libaxon_pjrt.so"
    holder = {}
    mod = types.ModuleType("antenv.axon_hooks")
    mod.set_axon_ntff_profile_hook = lambda h: holder.__setitem__("h", h)
    mod.get_axon_ntff_profile_hook = lambda: holder.get("h")
    sys.modules["antenv.axon_hooks"] = mod
    try:
        import antenv
        antenv.axon_hooks = mod
    except ImportError:
        pass

    lib = ctypes.CDLL(so_path)
    if not hasattr(lib, "axon_start_nrt_profile"):
        return
    lib.axon_start_nrt_profile.argtypes = [
        ctypes.POINTER(ctypes.c_int64), ctypes.c_size_t]
    lib.axon_start_nrt_profile.restype = ctypes.c_int64
    lib.axon_stop_nrt_profile.argtypes = [ctypes.c_char_p]
    lib.axon_stop_nrt_profile.restype = ctypes.c_int64

    @contextlib.contextmanager
    def _hook(output_dir, device_ids):
        import jax
        jax.devices()
        if device_ids:
            ids = (ctypes.c_int64 * len(device_ids))(*device_ids)
            rc = lib.axon_start_nrt_profile(ids, len(device_ids))
        else:
            rc = lib.axon_start_nrt_profile(None, 0)
        if rc != 0:
            raise RuntimeError(f"axon_start_nrt_profile rc={rc}")
        try:
            yield
        finally:
            n = lib.axon_stop_nrt_profile(str(output_dir).encode())
            print(f"profile: {n} file(s) written to {output_dir}")

    mod.set_axon_ntff_profile_hook(_hook)


def run(x, weight, bias, dec_w, dec_b, trace=False):
    from concourse import bass_utils
    from concourse.bass_utils import run_bass_kernel_spmd

    if trace:
        _install_ntff_hook()
        bass_utils.upload_artifacts = lambda tmpdir: tmpdir

    nc = _get_nc()
    in_maps = make_in_maps(x, weight, bias, dec_w)
    r = run_bass_kernel_spmd(nc, in_maps, list(range(NCORES)), trace=trace)
    return combine(r.results, dec_b), r


def kernel(x, weight, bias, dec_w, dec_b):
    out, _ = run(x, weight, bias, dec_w, dec_b, trace=False)
    return out


# revision 21
# speedup vs baseline: 1.0166x; 1.0166x over previous
"""Trainium2 Bass kernel for nn_LCN (locally-connected network).

Computation:
  x: (512, 1, 280, 280) -> non-overlapping 28x28 patches (10x10 grid, P=100)
  y[b, f, p] = sum_q x[b, p, q] * w[f*100+p, q]    (q = k*28+l, 784 per patch)
  y = relu(y + bias[f*100+p]);  out = y_flat @ dec_w.T + dec_b  (j = f*100+p)

Sharding: patch-parallel. All 8 cores see all 512 images; core c owns 13
patches (cores 4-7 own 12 real + 1 zero patch so every core runs the same
program). Per core:
  - host stages x TRANSPOSED as xT [128 px, 80 chunks, 512 b] bf16
    (im2col + transpose + cast all on host; DMA reads are contiguous
    multi-KB runs per partition at full HBM bandwidth)
  - conv: one matmul per (128-px chunk, patch-pair window): lhsT = staged
    weight tile [128, 32], rhs = xT[:, t, :] [128, 512] -> PSUM [128, 512]
    accumulating per 8-patch group (partition j = 16*local_patch + f)
  - ACT: relu(psum + bias) -> y_sb
  - decoder: 2 accumulating matmuls lhsT=dec [K, 10] -> out [10, 512]
Host sums the 8 per-core partial decoder outputs and adds dec_b.
"""

import sys

import numpy as np

for _p in ("/opt/trn_rl_repo", "/opt/trn_rl_repo/concourse"):
    if _p not in sys.path:
        sys.path.insert(0, _p)

import concourse.bass as bass
import concourse.mybir as mybir
import concourse.tile as tile
from concourse import bacc

F32 = mybir.dt.float32
BF16 = mybir.dt.bfloat16

# Problem constants
B = 512
P = 100
F = 16
OUT = 10
PPX = 784            # pixels per patch (28*28)
NCORES = 8

NPAT = 13            # patches per core program (zero-padded on 12-patch cores)
NCHUNK = 80          # ceil(13*784/128)
PXPAD = NCHUNK * 128  # 10240
GROUPS = [(0, 8), (8, 5)]   # (first local patch, n patches incl virtual pad)
GROUP_CHUNKS = [(0, 49), (49, 31)]  # (first chunk, n chunks); 8*784 = 49*128
# small first split to warm up PE early, small last split to trim the tail
_SPLIT_SIZES = [4, 8, 12, 12, 12, 12, 14, 6]
DMA_SPLITS = []
_c = 0
for _s in _SPLIT_SIZES:
    DMA_SPLITS.append((_c, _s))
    _c += _s
assert _c == NCHUNK

# per-core real patch ranges (cores 0-3: 13 patches, cores 4-7: 12)
CORE_PSTART = [0, 13, 26, 39, 52, 64, 76, 88]
CORE_NPAT = [13, 13, 13, 13, 12, 12, 12, 12]


def conv_plan():
    """Static matmul plan: one entry per (chunk, patch-pair window)."""
    plan = []
    for t in range(NCHUNK):
        p0 = (128 * t) // PPX
        p1 = (128 * t + 127) // PPX
        g = p0 // 8
        pairs = sorted({(min(p, 8 * g + 9) - 8 * g) // 2 for p in (p0, p1)})
        for k in pairs:
            first = ((8 * g + 2 * k) * PPX) // 128
            last = min(((8 * g + 2 * k + 2) * PPX - 1) // 128, NCHUNK - 1)
            plan.append((t, g, k, t == first, t == last))
    return plan

PLAN = conv_plan()
NMM = len(PLAN)  # 85


def build_program():
    nc = bacc.Bacc("TRN2")
    x_d = nc.dram_tensor("x", [128, NCHUNK * B], BF16, kind="ExternalInput")
    # w tiles then decoder staged in one tensor / one DMA
    w_d = nc.dram_tensor("w", [128, NMM * 32 + 2 * OUT], BF16,
                         kind="ExternalInput")
    b_d = nc.dram_tensor("bias", [128, 2], F32, kind="ExternalInput")
    o_d = nc.dram_tensor("out", [OUT, B], F32, kind="ExternalOutput")

    with tile.TileContext(nc) as tc:
        with (
            tc.tile_pool(name="const", bufs=1) as constp,
            tc.tile_pool(name="yps", bufs=2, space="PSUM") as ypsp,
            tc.tile_pool(name="wps", bufs=1, space="PSUM") as wpsp,
            tc.tile_pool(name="ops", bufs=1, space="PSUM") as opsp,
        ):
            w_sb = constp.tile([128, NMM * 32 + 2 * OUT], BF16)
            nc.scalar.dma_start(out=w_sb[:], in_=w_d[:])
            dec_sb = w_sb[:, NMM * 32:]
            bias_sb = constp.tile([128, 2], F32)
            nc.scalar.dma_start(out=bias_sb[:], in_=b_d[:])

            xt = constp.tile([128, NCHUNK, B], BF16)
            for c0, n in DMA_SPLITS:
                nc.sync.dma_start(
                    out=xt[:, c0:c0 + n, :],
                    in_=x_d[:, c0 * B:(c0 + n) * B],
                )

            # PE clock warm-up during the DMA pipe-fill window: M=128 dummy
            # matmuls on an uninitialized scratch tile (no input dependency,
            # output never read)
            warm_sb = constp.tile([128, 128 + B], BF16)
            nc.gpsimd.memset(warm_sb[:], 0.0)
            warm_ps = wpsp.tile([128, B], F32)
            for _ in range(9):
                nc.tensor.matmul(
                    warm_ps[:],
                    warm_sb[:, 0:128],
                    warm_sb[:, 128:],
                    start=True,
                    stop=True,
                )

            y_sb = constp.tile([128, 2, B], BF16)
            ps = [ypsp.tile([128, B], F32, name=f"ps{g}") for g in range(2)]
            out_ps = opsp.tile([OUT, B], F32)

            rows = [16 * 8, 16 * 5]  # evacuated rows per group
            out_sb = constp.tile([OUT, B], F32)
            for g in range(2):
                gi = [e for e in enumerate(PLAN) if e[1][1] == g]
                for i, (t, _, k, st, sp) in gi:
                    nc.tensor.matmul(
                        ps[g][32 * k:32 * k + 32, :],
                        w_sb[:, 32 * i:32 * i + 32],
                        xt[:, t, :],
                        start=st,
                        stop=sp,
                        tile_position=(0, 32 * k),
                    )
                nc.scalar.activation(
                    out=y_sb[0:rows[g], g, :],
                    in_=ps[g][0:rows[g], :],
                    func=mybir.ActivationFunctionType.Relu,
                    bias=bias_sb[0:rows[g], g:g + 1],
                )
                if g == 0:
                    nc.tensor.matmul(
                        out_ps[:],
                        dec_sb[0:rows[g], 0:OUT],
                        y_sb[0:rows[g], 0, :],
                        start=True,
                        stop=False,
                    )
                else:
                    # column-split the final decode/copy/store so DVE copy
                    # and output DMA overlap the second half's decode
                    h = B // 2
                    for c in range(2):
                        nc.tensor.matmul(
                            out_ps[:, c * h:(c + 1) * h],
                            dec_sb[0:rows[g], OUT:2 * OUT],
                            y_sb[0:rows[g], g, c * h:(c + 1) * h],
                            start=False,
                            stop=True,
                        )
                        nc.vector.tensor_copy(
                            out_sb[:, c * h:(c + 1) * h],
                            out_ps[:, c * h:(c + 1) * h])
                        nc.sync.dma_start(
                            out=o_d[:, c * h:(c + 1) * h],
                            in_=out_sb[:, c * h:(c + 1) * h])

    return nc


def stage_core(core, x_pm, weight, bias, dec_w):
    """Host-side staging for one core. x_pm: (B, 100, 784) float32."""
    import ml_dtypes

    p0 = CORE_PSTART[core]
    npr = CORE_NPAT[core]
    pids = list(range(p0, p0 + npr))

    xs = np.zeros((B, PXPAD), np.float32)
    xs[:, :npr * PPX] = x_pm[:, p0:p0 + npr, :].reshape(B, npr * PPX)
    # host-side transpose to [px_part 128, chunk, batch], bf16
    xs = np.ascontiguousarray(
        xs.reshape(B, NCHUNK, 128).transpose(2, 1, 0)
    ).astype(ml_dtypes.bfloat16).reshape(128, NCHUNK * B)

    wr = np.asarray(weight, np.float32).reshape(F, P, PPX)
    w_big = np.zeros((128, NMM * 32), np.float32)
    for i, (t, g, k, _, _) in enumerate(PLAN):
        for r in range(128):
            px = 128 * t + r
            p = px // PPX
            if p >= npr:
                continue
            pl = p - 8 * g
            if pl < 0 or pl // 2 != k:
                continue
            q = px % PPX
            w_big[r, 32 * i + (pl % 2) * 16:32 * i + (pl % 2) * 16 + F] = \
                wr[:, pids[p], q]

    br = np.asarray(bias, np.float32).reshape(F, P)
    dr = np.asarray(dec_w, np.float32).reshape(OUT, F, P)
    b_st = np.zeros((128, 2), np.float32)
    d_st = np.zeros((128, 2 * OUT), np.float32)
    for p in range(npr):
        g, pl = p // 8, p % 8
        j = 16 * pl + np.arange(F)
        b_st[j, g] = br[:, pids[p]]
        d_st[j[:, None], g * OUT + np.arange(OUT)[None, :]] = dr[:, :, pids[p]].T
    w_all = np.concatenate(
        [w_big, d_st], axis=1).astype(ml_dtypes.bfloat16)
    return {"x": xs, "w": w_all, "bias": b_st}


_cache = {}


def _get_nc():
    if "nc" not in _cache:
        nc = build_program()
        nc.finalize()
        _cache["nc"] = nc
    return _cache["nc"]


def make_in_maps(x, weight, bias, dec_w):
    x = np.asarray(x, np.float32)
    # patch-major pixel order: (b, ph, pw, k, l)
    x_pm = np.ascontiguousarray(
        x.reshape(B, 10, 28, 10, 28).transpose(0, 1, 3, 2, 4)
    ).reshape(B, P, PPX)
    return [stage_core(c, x_pm, weight, bias, dec_w) for c in range(NCORES)]


def combine(results, dec_b):
    acc = np.zeros((OUT, B), np.float32)
    for r in results:
        acc += r["out"]
    return acc.T + np.asarray(dec_b, np.float32)


def _install_ntff_hook():
    """Provide the missing antenv.axon_hooks module so trace=True works
    under axon (replicates trn_boot._ntff_profile_via_ctypes)."""
    import contextlib
    import ctypes
    import types

    if "antenv.axon_hooks" in sys.modules:
        return
    so_path = "/opt/axon/libaxon_pjrt.so"
    holder = {}
    mod = types.ModuleType("antenv.axon_hooks")
    mod.set_axon_ntff_profile_hook = lambda h: holder.__setitem__("h", h)
    mod.get_axon_ntff_profile_hook = lambda: holder.get("h")
    sys.modules["antenv.axon_hooks"] = mod
    try:
        import antenv
        antenv.axon_hooks = mod
    except ImportError:
        pass

    lib = ctypes.CDLL(so_path)
    if not hasattr(lib, "axon_start_nrt_profile"):
        return
    lib.axon_start_nrt_profile.argtypes = [
        ctypes.POINTER(ctypes.c_int64), ctypes.c_size_t]
    lib.axon_start_nrt_profile.restype = ctypes.c_int64
    lib.axon_stop_nrt_profile.argtypes = [ctypes.c_char_p]
    lib.axon_stop_nrt_profile.restype = ctypes.c_int64

    @contextlib.contextmanager
    def _hook(output_dir, device_ids):
        import jax
        jax.devices()
        if device_ids:
            ids = (ctypes.c_int64 * len(device_ids))(*device_ids)
            rc = lib.axon_start_nrt_profile(ids, len(device_ids))
        else:
            rc = lib.axon_start_nrt_profile(None, 0)
        if rc != 0:
            raise RuntimeError(f"axon_start_nrt_profile rc={rc}")
        try:
            yield
        finally:
            n = lib.axon_stop_nrt_profile(str(output_dir).encode())
            print(f"profile: {n} file(s) written to {output_dir}")

    mod.set_axon_ntff_profile_hook(_hook)


def run(x, weight, bias, dec_w, dec_b, trace=False):
    from concourse import bass_utils
    from concourse.bass_utils import run_bass_kernel_spmd

    if trace:
        _install_ntff_hook()
        bass_utils.upload_artifacts = lambda tmpdir: tmpdir

    nc = _get_nc()
    in_maps = make_in_maps(x, weight, bias, dec_w)
    r = run_bass_kernel_spmd(nc, in_maps, list(range(NCORES)), trace=trace)
    return combine(r.results, dec_b), r


def kernel(x, weight, bias, dec_w, dec_b):
    out, _ = run(x, weight, bias, dec_w, dec_b, trace=False)
    return out


# revision 22
# speedup vs baseline: 1.0182x; 1.0016x over previous
"""Trainium2 Bass kernel for nn_LCN (locally-connected network).

Computation:
  x: (512, 1, 280, 280) -> non-overlapping 28x28 patches (10x10 grid, P=100)
  y[b, f, p] = sum_q x[b, p, q] * w[f*100+p, q]    (q = k*28+l, 784 per patch)
  y = relu(y + bias[f*100+p]);  out = y_flat @ dec_w.T + dec_b  (j = f*100+p)

Sharding: patch-parallel. All 8 cores see all 512 images; core c owns 13
patches (cores 4-7 own 12 real + 1 zero patch so every core runs the same
program). Per core:
  - host stages x TRANSPOSED as xT [128 px, 80 chunks, 512 b] bf16
    (im2col + transpose + cast all on host; DMA reads are contiguous
    multi-KB runs per partition at full HBM bandwidth)
  - conv: one matmul per (128-px chunk, patch-pair window): lhsT = staged
    weight tile [128, 32], rhs = xT[:, t, :] [128, 512] -> PSUM [128, 512]
    accumulating per 8-patch group (partition j = 16*local_patch + f)
  - ACT: relu(psum + bias) -> y_sb
  - decoder: 2 accumulating matmuls lhsT=dec [K, 10] -> out [10, 512]
Host sums the 8 per-core partial decoder outputs and adds dec_b.
"""

import sys

import numpy as np

for _p in ("/opt/trn_rl_repo", "/opt/trn_rl_repo/concourse"):
    if _p not in sys.path:
        sys.path.insert(0, _p)

import concourse.bass as bass
import concourse.mybir as mybir
import concourse.tile as tile
from concourse import bacc

F32 = mybir.dt.float32
BF16 = mybir.dt.bfloat16

# Problem constants
B = 512
P = 100
F = 16
OUT = 10
PPX = 784            # pixels per patch (28*28)
NCORES = 8

NPAT = 13            # patches per core program (zero-padded on 12-patch cores)
NCHUNK = 80          # ceil(13*784/128)
PXPAD = NCHUNK * 128  # 10240
GROUPS = [(0, 8), (8, 5)]   # (first local patch, n patches incl virtual pad)
GROUP_CHUNKS = [(0, 49), (49, 31)]  # (first chunk, n chunks); 8*784 = 49*128
# small first split to warm up PE early, small last split to trim the tail
_SPLIT_SIZES = [4, 8, 10, 10, 10, 10, 10, 8, 6, 4]
DMA_SPLITS = []
_c = 0
for _s in _SPLIT_SIZES:
    DMA_SPLITS.append((_c, _s))
    _c += _s
assert _c == NCHUNK

# per-core real patch ranges (cores 0-3: 13 patches, cores 4-7: 12)
CORE_PSTART = [0, 13, 26, 39, 52, 64, 76, 88]
CORE_NPAT = [13, 13, 13, 13, 12, 12, 12, 12]


def conv_plan():
    """Static matmul plan: one entry per (chunk, patch-pair window)."""
    plan = []
    for t in range(NCHUNK):
        p0 = (128 * t) // PPX
        p1 = (128 * t + 127) // PPX
        g = p0 // 8
        pairs = sorted({(min(p, 8 * g + 9) - 8 * g) // 2 for p in (p0, p1)})
        for k in pairs:
            first = ((8 * g + 2 * k) * PPX) // 128
            last = min(((8 * g + 2 * k + 2) * PPX - 1) // 128, NCHUNK - 1)
            plan.append((t, g, k, t == first, t == last))
    return plan

PLAN = conv_plan()
NMM = len(PLAN)  # 85


def build_program():
    nc = bacc.Bacc("TRN2")
    x_d = nc.dram_tensor("x", [128, NCHUNK * B], BF16, kind="ExternalInput")
    # w tiles then decoder staged in one tensor / one DMA
    w_d = nc.dram_tensor("w", [128, NMM * 32 + 2 * OUT], BF16,
                         kind="ExternalInput")
    b_d = nc.dram_tensor("bias", [128, 2], F32, kind="ExternalInput")
    o_d = nc.dram_tensor("out", [OUT, B], F32, kind="ExternalOutput")

    with tile.TileContext(nc) as tc:
        with (
            tc.tile_pool(name="const", bufs=1) as constp,
            tc.tile_pool(name="yps", bufs=2, space="PSUM") as ypsp,
            tc.tile_pool(name="wps", bufs=1, space="PSUM") as wpsp,
            tc.tile_pool(name="ops", bufs=1, space="PSUM") as opsp,
        ):
            w_sb = constp.tile([128, NMM * 32 + 2 * OUT], BF16)
            nc.scalar.dma_start(out=w_sb[:], in_=w_d[:])
            dec_sb = w_sb[:, NMM * 32:]
            bias_sb = constp.tile([128, 2], F32)
            nc.scalar.dma_start(out=bias_sb[:], in_=b_d[:])

            xt = constp.tile([128, NCHUNK, B], BF16)
            for c0, n in DMA_SPLITS:
                nc.sync.dma_start(
                    out=xt[:, c0:c0 + n, :],
                    in_=x_d[:, c0 * B:(c0 + n) * B],
                )

            # PE clock warm-up during the DMA pipe-fill window: M=128 dummy
            # matmuls on an uninitialized scratch tile (no input dependency,
            # output never read)
            warm_sb = constp.tile([128, 128 + B], BF16)
            nc.gpsimd.memset(warm_sb[:], 0.0)
            warm_ps = wpsp.tile([128, B], F32)
            for _ in range(9):
                nc.tensor.matmul(
                    warm_ps[:],
                    warm_sb[:, 0:128],
                    warm_sb[:, 128:],
                    start=True,
                    stop=True,
                )

            y_sb = constp.tile([128, 2, B], BF16)
            ps = [ypsp.tile([128, B], F32, name=f"ps{g}") for g in range(2)]
            out_ps = opsp.tile([OUT, B], F32)

            rows = [16 * 8, 16 * 5]  # evacuated rows per group
            out_sb = constp.tile([OUT, B], F32)
            for g in range(2):
                gi = [e for e in enumerate(PLAN) if e[1][1] == g]
                for i, (t, _, k, st, sp) in gi:
                    nc.tensor.matmul(
                        ps[g][32 * k:32 * k + 32, :],
                        w_sb[:, 32 * i:32 * i + 32],
                        xt[:, t, :],
                        start=st,
                        stop=sp,
                        tile_position=(0, 32 * k),
                    )
                nc.scalar.activation(
                    out=y_sb[0:rows[g], g, :],
                    in_=ps[g][0:rows[g], :],
                    func=mybir.ActivationFunctionType.Relu,
                    bias=bias_sb[0:rows[g], g:g + 1],
                )
                if g == 0:
                    nc.tensor.matmul(
                        out_ps[:],
                        dec_sb[0:rows[g], 0:OUT],
                        y_sb[0:rows[g], 0, :],
                        start=True,
                        stop=False,
                    )
                else:
                    # column-split the final decode/copy/store so DVE copy
                    # and output DMA overlap the second half's decode
                    h = B // 2
                    for c in range(2):
                        nc.tensor.matmul(
                            out_ps[:, c * h:(c + 1) * h],
                            dec_sb[0:rows[g], OUT:2 * OUT],
                            y_sb[0:rows[g], g, c * h:(c + 1) * h],
                            start=False,
                            stop=True,
                        )
                        nc.vector.tensor_copy(
                            out_sb[:, c * h:(c + 1) * h],
                            out_ps[:, c * h:(c + 1) * h])
                        nc.sync.dma_start(
                            out=o_d[:, c * h:(c + 1) * h],
                            in_=out_sb[:, c * h:(c + 1) * h])

    return nc


def stage_core(core, x_pm, weight, bias, dec_w):
    """Host-side staging for one core. x_pm: (B, 100, 784) float32."""
    import ml_dtypes

    p0 = CORE_PSTART[core]
    npr = CORE_NPAT[core]
    pids = list(range(p0, p0 + npr))

    xs = np.zeros((B, PXPAD), np.float32)
    xs[:, :npr * PPX] = x_pm[:, p0:p0 + npr, :].reshape(B, npr * PPX)
    # host-side transpose to [px_part 128, chunk, batch], bf16
    xs = np.ascontiguousarray(
        xs.reshape(B, NCHUNK, 128).transpose(2, 1, 0)
    ).astype(ml_dtypes.bfloat16).reshape(128, NCHUNK * B)

    wr = np.asarray(weight, np.float32).reshape(F, P, PPX)
    w_big = np.zeros((128, NMM * 32), np.float32)
    for i, (t, g, k, _, _) in enumerate(PLAN):
        for r in range(128):
            px = 128 * t + r
            p = px // PPX
            if p >= npr:
                continue
            pl = p - 8 * g
            if pl < 0 or pl // 2 != k:
                continue
            q = px % PPX
            w_big[r, 32 * i + (pl % 2) * 16:32 * i + (pl % 2) * 16 + F] = \
                wr[:, pids[p], q]

    br = np.asarray(bias, np.float32).reshape(F, P)
    dr = np.asarray(dec_w, np.float32).reshape(OUT, F, P)
    b_st = np.zeros((128, 2), np.float32)
    d_st = np.zeros((128, 2 * OUT), np.float32)
    for p in range(npr):
        g, pl = p // 8, p % 8
        j = 16 * pl + np.arange(F)
        b_st[j, g] = br[:, pids[p]]
        d_st[j[:, None], g * OUT + np.arange(OUT)[None, :]] = dr[:, :, pids[p]].T
    w_all = np.concatenate(
        [w_big, d_st], axis=1).astype(ml_dtypes.bfloat16)
    return {"x": xs, "w": w_all, "bias": b_st}


_cache = {}


def _get_nc():
    if "nc" not in _cache:
        nc = build_program()
        nc.finalize()
        _cache["nc"] = nc
    return _cache["nc"]


def make_in_maps(x, weight, bias, dec_w):
    x = np.asarray(x, np.float32)
    # patch-major pixel order: (b, ph, pw, k, l)
    x_pm = np.ascontiguousarray(
        x.reshape(B, 10, 28, 10, 28).transpose(0, 1, 3, 2, 4)
    ).reshape(B, P, PPX)
    return [stage_core(c, x_pm, weight, bias, dec_w) for c in range(NCORES)]


def combine(results, dec_b):
    acc = np.zeros((OUT, B), np.float32)
    for r in results:
        acc += r["out"]
    return acc.T + np.asarray(dec_b, np.float32)


def _install_ntff_hook():
    """Provide the missing antenv.axon_hooks module so trace=True works
    under axon (replicates trn_boot._ntff_profile_via_ctypes)."""
    import contextlib
    import ctypes
    import types

    if "antenv.axon_hooks" in sys.modules:
        return
    so_path = "/opt/axon/libaxon_pjrt.so"
    holder = {}
    mod = types.ModuleType("antenv.axon_hooks")
    mod.set_axon_ntff_profile_hook = lambda h: holder.__setitem__("h", h)
    mod.get_axon_ntff_profile_hook = lambda: holder.get("h")
    sys.modules["antenv.axon_hooks"] = mod
    try:
        import antenv
        antenv.axon_hooks = mod
    except ImportError:
        pass

    lib = ctypes.CDLL(so_path)
    if not hasattr(lib, "axon_start_nrt_profile"):
        return
    lib.axon_start_nrt_profile.argtypes = [
        ctypes.POINTER(ctypes.c_int64), ctypes.c_size_t]
    lib.axon_start_nrt_profile.restype = ctypes.c_int64
    lib.axon_stop_nrt_profile.argtypes = [ctypes.c_char_p]
    lib.axon_stop_nrt_profile.restype = ctypes.c_int64

    @contextlib.contextmanager
    def _hook(output_dir, device_ids):
        import jax
        jax.devices()
        if device_ids:
            ids = (ctypes.c_int64 * len(device_ids))(*device_ids)
            rc = lib.axon_start_nrt_profile(ids, len(device_ids))
        else:
            rc = lib.axon_start_nrt_profile(None, 0)
        if rc != 0:
            raise RuntimeError(f"axon_start_nrt_profile rc={rc}")
        try:
            yield
        finally:
            n = lib.axon_stop_nrt_profile(str(output_dir).encode())
            print(f"profile: {n} file(s) written to {output_dir}")

    mod.set_axon_ntff_profile_hook(_hook)


def run(x, weight, bias, dec_w, dec_b, trace=False):
    from concourse import bass_utils
    from concourse.bass_utils import run_bass_kernel_spmd

    if trace:
        _install_ntff_hook()
        bass_utils.upload_artifacts = lambda tmpdir: tmpdir

    nc = _get_nc()
    in_maps = make_in_maps(x, weight, bias, dec_w)
    r = run_bass_kernel_spmd(nc, in_maps, list(range(NCORES)), trace=trace)
    return combine(r.results, dec_b), r


def kernel(x, weight, bias, dec_w, dec_b):
    out, _ = run(x, weight, bias, dec_w, dec_b, trace=False)
    return out
